# revision 23
# baseline (speedup 1.0000x reference)
"""GNN message-passing kernel for 8 TRN2 NeuronCores.

Strategy (edges sharded by destination, nodes RELABELED for load balance):
  reference per hop:
    messages = concat(h[src], h[dst]) @ W_msg + b_msg          [E, D]
    agg      = segment_sum(messages, dst)                      [N, D]
    h        = relu(concat(h, agg) @ W_upd + b_upd)            [N, D]
  Algebraic reduction (exact):
    agg = S @ Wm_top + (deg * h) @ Wm_bot + outer(deg, b_msg),
    where S = segment_sum(h[src], dst)  -- pure gather+segment-sum.
  Folding agg through the update GEMM (host-precomputed weight products):
    h_new = relu(h @ U_top + S @ A + (deg*h) @ B + outer(deg, c) + b_upd)
    A = Wm_top @ U_bot,  B = Wm_bot @ U_bot,  c = b_msg @ U_bot.
  This removes the O(E*D^2) edge GEMM entirely.

  Load balance: nodes are relabeled host-side (LPT bin packing by in-degree)
  so every 128-dst tile has <=2048 in-edges -> p_tile=16 exactly, no
  stragglers.  Padding gather slots use spread indices (never one hot row).

  Per hop each core gathers h[src] rows for its ~E/8 edges with dma_gather
  (two half-tile gathers per dst tile, round-robined over the 4 SWDGE
  queues to keep all 4 Q7 descriptor-gen pairs busy), segment-sums them
  with 0/1-indicator matmuls on the PE, then runs the fused node-update
  GEMM on its N/8 nodes.

  Node-state exchange: the relabeling puts each core's tiles 0-19 in table
  rows [0,20480) and tiles 20-24 in [20480,25600), so the per-hop exchange
  is TWO shared-output AllGathers: the big one (tiles 0-19) is issued as
  soon as block 4 of the update loop finishes and overlaps the rest of the
  hop; only the small tail AllGather (5 tiles) is exposed at the hop
  boundary.
"""
import numpy as np
import heapq
import ml_dtypes

import concourse.bacc as bacc
import concourse.mybir as mybir
import concourse.tile as tile
from concourse import bass_utils
from concourse.bass import _add_dep_helper

# ---- problem constants (hardcoded per contract) ----
N = 25000
E = 400000
D = 256
F = 32
HOPS = 4
NTYPES = 10
NC_ = 8               # cores
PER = 3200            # dst rows per core (25600 padded)
NPAD = NC_ * PER      # 25600
TILES = PER // 128    # 25 dst tiles per core
TA = 20               # tiles in the early (big) AllGather chunk
TB = TILES - TA       # tiles in the tail AllGather chunk
ROWS_A = TA * 128     # 2560
ROWS_B = TB * 128     # 640

BF16 = ml_dtypes.bfloat16
_nc_cache = {}


def _balance_tiles(dst):
    """LPT bin packing: assign each node to one of 200 tiles (<=128 nodes
    each) equalizing total in-degree per tile."""
    NT = NC_ * TILES
    deg = np.bincount(dst, minlength=N)
    order = np.argsort(-deg, kind="stable")
    heap = [(0, 0, t) for t in range(NT)]
    heapq.heapify(heap)
    sums = np.zeros(NT, np.int64)
    cnts = np.zeros(NT, np.int64)
    assign = np.empty(N, np.int64)
    pos = np.empty(N, np.int64)
    for n in order:
        while True:
            s, c, t = heapq.heappop(heap)
            if cnts[t] < 128 and sums[t] == s and cnts[t] == c:
                break
        assign[n] = t
        pos[n] = cnts[t]
        sums[t] += deg[n]
        cnts[t] += 1
        if cnts[t] < 128:
            heapq.heappush(heap, (sums[t], cnts[t], t))
    return assign, pos, sums


def _gid(t_global, p):
    """Global table row for tile t_global (=c*TILES+tpos), slot p, under the
    A/B-region relabeling (tiles 0..TA-1 -> region A, rest -> region B)."""
    c, t = t_global // TILES, t_global % TILES
    a = c * ROWS_A + t * 128 + p
    b = NC_ * ROWS_A + c * ROWS_B + (t - TA) * 128 + p
    if isinstance(t, np.ndarray):
        return np.where(t < TA, a, b)
    return a if t < TA else b


# ---------------- host-side preprocessing ----------------
def _prep(inputs):
    edges = np.asarray(inputs["edges"])
    src = edges[0].astype(np.int64)
    dst = edges[1].astype(np.int64)

    assign, pos, tile_sums = _balance_tiles(dst)
    p_tile = int(np.ceil(tile_sums.max() / 128))
    tile_e = p_tile * 128
    e_pad = TILES * tile_e

    # new node id (table row) for every original node
    new_id = _gid(assign, pos)

    src_n = new_id[src]
    tile_of = assign[dst]           # destination tile (global 0..199)
    dloc_of = pos[dst]              # slot within destination tile

    order = np.argsort(tile_of, kind="stable")
    src_s = src_n[order]
    tile_s = tile_of[order]
    dloc_s = dloc_of[order]
    counts = np.bincount(tile_s, minlength=NC_ * TILES)
    starts = np.zeros(NC_ * TILES + 1, np.int64)
    np.cumsum(counts, out=starts[1:])

    # padding gather slots get spread-out (but valid) indices so they don't
    # all hammer the same HBM row
    pad_idx = (np.arange(e_pad, dtype=np.int64) * 67) % NPAD

    idx_arrs, dloc_arrs, deg_arrs = [], [], []
    deg_new = np.zeros(NPAD, np.int64)
    np.add.at(deg_new, new_id[dst], 1)
    for c in range(NC_):
        idx_c = pad_idx.astype(np.int16).copy()
        dl_c = np.full((TILES, 128, p_tile), 255.0, BF16)
        for t in range(TILES):
            g = c * TILES + t
            lo, hi = starts[g], starts[g + 1]
            n = hi - lo
            base = t * tile_e
            idx_c[base:base + n] = src_s[lo:hi].astype(np.int16)
            j = np.arange(n)
            dl_c[t, j % 128, j // 128] = dloc_s[lo:hi].astype(BF16)
        wrapped = idx_c.reshape(-1, 16).T
        idx_arr = np.zeros((128, e_pad // 16), np.int16)
        for k in range(8):
            idx_arr[16 * k:16 * (k + 1)] = wrapped
        idx_arrs.append(idx_arr)
        dloc_arrs.append(dl_c.transpose(1, 0, 2).reshape(128, TILES * p_tile))

        # degree of this core's LOCAL rows (local order = (t, p))
        loc = np.empty(PER, np.int64)
        for t in range(TILES):
            loc[t * 128:(t + 1) * 128] = _gid(c * TILES + t, np.arange(128))
        deg = deg_new[loc]
        deg_arrs.append(np.broadcast_to(deg.astype(np.float32), (128, PER)).astype(BF16))

    iota = np.broadcast_to((np.arange(tile_e) % 128).astype(BF16), (128, tile_e)).copy()

    # fused weights
    W_msg = np.asarray(inputs["W_msg"], np.float32)
    W_upd = np.asarray(inputs["W_upd"], np.float32)
    b_msg = np.asarray(inputs["b_msg"], np.float32)
    b_upd = np.asarray(inputs["b_upd"], np.float32)
    wf = np.zeros((HOPS, 6, 128, D), BF16)
    cvec = np.zeros((HOPS, 1, D), BF16)
    for i in range(HOPS):
        U_t = W_upd[i][:D]          # [256,256]
        U_b = W_upd[i][D:]
        A = W_msg[i][:D] @ U_b
        B = W_msg[i][D:] @ U_b
        for k in range(2):
            wf[i, 0 + k] = U_t[128 * k:128 * (k + 1)]
            wf[i, 2 + k] = A[128 * k:128 * (k + 1)]
            wf[i, 4 + k] = B[128 * k:128 * (k + 1)]
        cvec[i, 0] = b_msg[i] @ U_b
    b_upd_t = np.zeros((128, 2 * HOPS), np.float32)
    for i in range(HOPS):
        b_upd_t[:, 2 * i] = b_upd[i][:128]
        b_upd_t[:, 2 * i + 1] = b_upd[i][128:]

    # h0 ingredients, reindexed into each core's local (t, p) order
    nodes = np.asarray(inputs["nodes"], np.float32)
    node_types = np.asarray(inputs["node_types"], np.int64)
    type_emb_eff = np.asarray(inputs["type_emb"], np.float32) + np.asarray(inputs["b_proj"], np.float32)[None, :]
    te = np.zeros((16, D), BF16)
    te[:NTYPES] = type_emb_eff.astype(BF16)
    W_proj = np.asarray(inputs["W_proj"], np.float32).astype(BF16)  # [32,256]

    # tile -> ordered original-node list (pos-indexed), -1 for empty slots
    tile_nodes = np.full((NC_ * TILES, 128), -1, np.int64)
    tile_nodes[assign, pos] = np.arange(N)

    nodes_T, onehot, vmask = [], [], []
    for c in range(NC_):
        tn = tile_nodes[c * TILES:(c + 1) * TILES].reshape(PER)  # local order
        valid = tn >= 0
        tn_safe = np.where(valid, tn, 0)
        nt = np.where(valid[None, :], nodes[tn_safe].T, 0.0).astype(BF16)
        oh = np.zeros((16, PER), BF16)
        vsel = np.nonzero(valid)[0]
        oh[node_types[tn[vsel]], vsel] = 1.0
        vm = np.broadcast_to(valid.astype(np.float32), (128, PER)).astype(BF16)
        nodes_T.append(nt)
        onehot.append(oh)
        vmask.append(np.ascontiguousarray(vm))

    ident = np.eye(128, dtype=BF16)
    per_core = []
    for c in range(NC_):
        per_core.append(dict(
            idx=idx_arrs[c], dloc=dloc_arrs[c], iota=iota, degb=deg_arrs[c],
            nodesT=nodes_T[c], onehot=onehot[c], vmask=vmask[c],
            wf=wf, cvec=cvec, bupd=b_upd_t, te=te, wproj=W_proj, ident=ident,
        ))
    return per_core, p_tile


# ---------------- device kernel ----------------
def _build(p_tile):
    tile_e = p_tile * 128
    e_pad = TILES * tile_e
    half1 = (p_tile + 1) // 2       # columns in first half-gather
    half2 = p_tile - half1
    fp32 = mybir.dt.float32
    bf16 = mybir.dt.bfloat16

    nc = bacc.Bacc("TRN2", target_bir_lowering=False, debug=False,
                   enable_asserts=True, num_devices=NC_,
                   num_swdge_queues=4)
    # inputs
    idx_d = nc.dram_tensor("idx", [128, e_pad // 16], mybir.dt.int16, kind="ExternalInput")
    dloc_d = nc.dram_tensor("dloc", [128, TILES * p_tile], bf16, kind="ExternalInput")
    iota_d = nc.dram_tensor("iota", [128, tile_e], bf16, kind="ExternalInput")
    degb_d = nc.dram_tensor("degb", [128, PER], bf16, kind="ExternalInput")
    nodesT_d = nc.dram_tensor("nodesT", [F, PER], bf16, kind="ExternalInput")
    onehot_d = nc.dram_tensor("onehot", [16, PER], bf16, kind="ExternalInput")
    vmask_d = nc.dram_tensor("vmask", [128, PER], bf16, kind="ExternalInput")
    wf_d = nc.dram_tensor("wf", [HOPS, 6, 128, D], bf16, kind="ExternalInput")
    cvec_d = nc.dram_tensor("cvec", [HOPS, 1, D], bf16, kind="ExternalInput")
    bupd_d = nc.dram_tensor("bupd", [128, 2 * HOPS], fp32, kind="ExternalInput")
    te_d = nc.dram_tensor("te", [16, D], bf16, kind="ExternalInput")
    wproj_d = nc.dram_tensor("wproj", [F, D], bf16, kind="ExternalInput")
    ident_d = nc.dram_tensor("ident", [128, 128], bf16, kind="ExternalInput")
    # output
    locmax_d = nc.dram_tensor("locmax", [128, 2], fp32, kind="ExternalOutput")

    RELU = mybir.ActivationFunctionType.Relu
    COPY = mybir.ActivationFunctionType.Copy
    EQ = mybir.AluOpType.is_equal

    with tile.TileContext(nc) as tc:
        with (
            tc.tile_pool(name="dram", bufs=1, space="DRAM") as dram,
            tc.tile_pool(name="stat", bufs=1) as stat,
            tc.tile_pool(name="mstream", bufs=4) as mpool,
            tc.tile_pool(name="gpool", bufs=8) as gpool,
            tc.tile_pool(name="hT", bufs=2) as hTpool,
            tc.tile_pool(name="work", bufs=1) as work,
            tc.tile_pool(name="spsum", bufs=2, space="PSUM") as spsum,
            tc.tile_pool(name="tpsum", bufs=2, space="PSUM") as tpsum,
            tc.tile_pool(name="upsum", bufs=2, space="PSUM") as upsum,
        ):
            # static SBUF loads
            idx_t = stat.tile([128, e_pad // 16], mybir.dt.int16)
            nc.sync.dma_start(idx_t[:], idx_d[:])
            dloc_sb = stat.tile([128, TILES * p_tile], bf16)
            nc.sync.dma_start(dloc_sb[:], dloc_d[:])
            iota_sb = stat.tile([128, tile_e], bf16)
            nc.sync.dma_start(iota_sb[:], iota_d[:])
            degb = stat.tile([128, PER], bf16)
            nc.sync.dma_start(degb[:], degb_d[:])
            vmask = stat.tile([128, PER], bf16)
            nc.sync.dma_start(vmask[:], vmask_d[:])
            wf_sb = stat.tile([128, HOPS * 6 * D], bf16, name="wf_sb")
            nc.sync.dma_start(
                wf_sb[:].rearrange("p (h s d) -> p h s d", h=HOPS, s=6),
                wf_d.rearrange("h s p d -> p h s d"),
            )
            cvec_sb = stat.tile([1, HOPS * D], bf16)
            nc.sync.dma_start(
                cvec_sb[:].rearrange("o (h d) -> o h d", h=HOPS),
                cvec_d.rearrange("h o d -> o h d"),
            )
            bupd_sb = stat.tile([128, 2 * HOPS], fp32)
            nc.sync.dma_start(bupd_sb[:], bupd_d[:])
            te_sb = stat.tile([16, D], bf16)
            nc.sync.dma_start(te_sb[:], te_d[:])
            wproj_sb = stat.tile([F, D], bf16)
            nc.sync.dma_start(wproj_sb[:], wproj_d[:])
            ident = stat.tile([128, 128], bf16)
            nc.sync.dma_start(ident[:], ident_d[:])
            nodesT_sb = stat.tile([F, PER], bf16)
            nc.sync.dma_start(nodesT_sb[:], nodesT_d[:])
            onehot_sb = stat.tile([16, PER], bf16)
            nc.sync.dma_start(onehot_sb[:], onehot_d[:])

            # DRAM tables; rows [0, NC*ROWS_A) hold every core's tiles 0-19,
            # rows [NC*ROWS_A, NPAD) the tail tiles 20-24.  A and B regions
            # are separate Shared tensors (the sim enforces one writer per
            # Shared tensor) allocated back-to-back so the gather can address
            # the whole [NPAD, D] table from tabA's base.
            tabsA, tabsB = [], []
            global _tab_pairs
            _tab_pairs = []
            for i in range(HOPS):
                tabsA.append(dram.tile([NC_ * ROWS_A, D], bf16, addr_space="Shared",
                                       name=f"htabA{i}", tag=f"htabA{i}"))
                tabsB.append(dram.tile([NC_ * ROWS_B, D], bf16, addr_space="Shared",
                                       name=f"htabB{i}", tag=f"htabB{i}"))
                _tab_pairs.append((tabsA[i], tabsB[i]))
            bncA = [dram.tile([ROWS_A, D], bf16, name=f"bncA{i}", tag=f"bncA{i}")
                    for i in range(HOPS)]
            bncB = [dram.tile([ROWS_B, D], bf16, name=f"bncB{i}", tag=f"bncB{i}")
                    for i in range(HOPS)]

            def ag_chunk(i, row_tile, which):
                """AllGather one region of the hop-i node table."""
                if which == 0:
                    bnc, out_ap = bncA[i], tabsA[i][:, :]
                else:
                    bnc, out_ap = bncB[i], tabsB[i][:, :]
                with tc.high_priority():
                    nc.sync.dma_start(
                        bnc.rearrange("(t p) f -> p t f", p=128),
                        row_tile[:].rearrange("p (t f) -> p t f", f=256))
                    return nc.gpsimd.collective_compute(
                        "AllGather", mybir.AluOpType.bypass,
                        replica_groups=[list(range(NC_))],
                        ins=[bnc.opt()], outs=[out_ap.opt()],
                    )

            def wf_chunk(hop, s, fo):
                base = (hop * 6 + s) * D
                return wf_sb[:, base + fo * 128: base + (fo + 1) * 128]

            def make_m(t):
                # indicator matrix for tile t: m[e, p*128+d] = (dloc[e,p]==d)
                m_t = mpool.tile([128, tile_e], bf16, name="mt", tag="mt")
                nc.vector.tensor_tensor(
                    m_t[:].rearrange("q (p e) -> q p e", e=128),
                    iota_sb[:].rearrange("q (p e) -> q p e", e=128),
                    dloc_sb[:, t * p_tile:(t + 1) * p_tile]
                        .unsqueeze(2).broadcast_to([128, p_tile, 128]),
                    op=EQ)
                return m_t

            # ---- h0 phase: local shard only ----
            hT0 = hTpool.tile([128, PER], bf16, name="hTa", tag="hTa")
            hT1 = hTpool.tile([128, PER], bf16, name="hTb", tag="hTb")
            row_stageA = work.tile([128, TA * 256], bf16, name="rowstageA")
            row_stageB = work.tile([128, TB * 256], bf16, name="rowstageB")
            ccs = []
            for t in range(TILES):
                stage, toff = ((row_stageA, t) if t < TA
                               else (row_stageB, t - TA))
                ps = upsum.tile([128, 512], fp32, name="ups", tag="ups")
                nc.tensor.matmul(ps[:, :D], nodesT_sb[:, t * 128:(t + 1) * 128],
                                 wproj_sb[:], start=True, stop=False)
                nc.tensor.matmul(ps[:, :D], onehot_sb[:, t * 128:(t + 1) * 128],
                                 te_sb[:], start=False, stop=True)
                nc.scalar.activation(stage[:, toff * 256:(toff + 1) * 256], ps[:, :D], COPY)
                for fo in range(2):
                    tp = tpsum.tile([128, 128], bf16, name="tp", tag="tp")
                    nc.tensor.transpose(tp[:], stage[:, toff * 256 + fo * 128:toff * 256 + (fo + 1) * 128], ident[:])
                    dst = hT0 if fo == 0 else hT1
                    nc.vector.tensor_copy(dst[:, t * 128:(t + 1) * 128], tp[:])
                if t == TA - 1:
                    ccs.append(ag_chunk(0, row_stageA, 0))
            ccs.append(ag_chunk(0, row_stageB, 1))

            hT = [hT0, hT1]
            # ---- hops ----
            for i in range(HOPS):
                S_T0 = work.tile([128, PER], bf16, name="st0", tag="st0")
                S_T1 = work.tile([128, PER], bf16, name="st1", tag="st1")
                hdeg0 = work.tile([128, PER], bf16, name="hd0", tag="hd0")
                hdeg1 = work.tile([128, PER], bf16, name="hd1", tag="hd1")
                nc.vector.tensor_tensor(hdeg0[:], hT[0][:], degb[:], op=mybir.AluOpType.mult)
                nc.vector.tensor_tensor(hdeg1[:], hT[1][:], degb[:], op=mybir.AluOpType.mult)
                hTn0 = hTpool.tile([128, PER], bf16, name="hTa", tag="hTa")
                hTn1 = hTpool.tile([128, PER], bf16, name="hTb", tag="hTb")
                states = [hT[0], hT[1], S_T0, S_T1, hdeg0, hdeg1]
                last = i == HOPS - 1
                prev_ccs, ccs = ccs, []
                if not last:
                    row_stage2A = work.tile([128, TA * 256], bf16, name="rowstageA")
                    row_stage2B = work.tile([128, TB * 256], bf16, name="rowstageB")

                for b0 in range(0, PER, 512):
                    bs = min(512, PER - b0)
                    for t in range(b0 // 128, (b0 + bs) // 128):
                        # two half-tile gathers on different SWDGE queues
                        ghalves = []
                        for h, (c0, cn) in enumerate(((0, half1), (half1, half2))):
                            g = gpool.tile([128, cn, D], bf16, name=f"g{h}", tag=f"g{h}")
                            gi = nc.gpsimd.dma_gather(
                                g[:], tabsA[i][:, :],
                                idx_t[:, (t * tile_e + c0 * 128) // 16:
                                         (t * tile_e + (c0 + cn) * 128) // 16],
                                num_idxs=cn * 128, num_idxs_reg=cn * 128,
                                elem_size=D, single_packet=False,
                                queue_num=(2 * t + h) % 4,
                            )
                            for cc in prev_ccs:
                                _add_dep_helper(gi.ins, cc.ins, sync=True,
                                                reason="gather after table AllGather")
                            if ccs:
                                # order-only: keep the gpsimd queue slot for
                                # this hop's early AllGather trigger ahead of
                                # the tail blocks' gathers so it fires as soon
                                # as its bounce is ready and overlaps them
                                _add_dep_helper(gi.ins, ccs[0].ins, sync=False,
                                                reason="late gathers after early AG trigger")
                            ghalves.append((g, cn))
                        m_t = make_m(t)
                        sp = spsum.tile([128, D], fp32, name="sp", tag="sp")
                        pacc = 0
                        for g, cn in ghalves:
                            for p in range(cn):
                                nc.tensor.matmul(sp[:], m_t[:, (pacc + p) * 128:(pacc + p + 1) * 128],
                                                 g[:, p, :], start=(pacc + p == 0),
                                                 stop=(pacc + p == p_tile - 1))
                            pacc += cn
                        s_sb = work.tile([128, D], bf16, name="ssb", tag="ssb", bufs=2)
                        nc.scalar.activation(s_sb[:], sp[:], COPY)
                        for fo in range(2):
                            tp = tpsum.tile([128, 128], bf16, name="tp", tag="tp")
                            nc.tensor.transpose(tp[:], s_sb[:, fo * 128:(fo + 1) * 128], ident[:])
                            dst = S_T0 if fo == 0 else S_T1
                            nc.vector.tensor_copy(dst[:, t * 128:(t + 1) * 128], tp[:])

                    for fo in range(2):
                        hTn = hTn0 if fo == 0 else hTn1
                        ps = upsum.tile([128, 512], fp32, name="ups", tag="ups")
                        for s in range(6):
                            nc.tensor.matmul(ps[:, :bs], wf_chunk(i, s, fo),
                                             states[s][:, b0:b0 + bs],
                                             start=(s == 0), stop=False)
                        nc.tensor.matmul(ps[:, :bs],
                                         cvec_sb[:, i * D + fo * 128: i * D + (fo + 1) * 128],
                                         degb[0:1, b0:b0 + bs],
                                         start=False, stop=True)
                        nc.scalar.activation(hTn[:, b0:b0 + bs], ps[:, :bs], RELU,
                                             bias=bupd_sb[:, 2 * i + fo: 2 * i + fo + 1])
                    # mask + row-major transposes for the finished block
                    nc.vector.tensor_tensor(hTn0[:, b0:b0 + bs], hTn0[:, b0:b0 + bs],
                                            vmask[:, b0:b0 + bs], op=mybir.AluOpType.mult)
                    nc.vector.tensor_tensor(hTn1[:, b0:b0 + bs], hTn1[:, b0:b0 + bs],
                                            vmask[:, b0:b0 + bs], op=mybir.AluOpType.mult)
                    if not last:
                        for t in range(b0 // 128, (b0 + bs) // 128):
                            stage, toff = ((row_stage2A, t) if t < TA
                                           else (row_stage2B, t - TA))
                            for fo in range(2):
                                srcT = hTn0 if fo == 0 else hTn1
                                tp = tpsum.tile([128, 128], bf16, name="tp", tag="tp")
                                nc.tensor.transpose(
                                    tp[:], srcT[:, t * 128:(t + 1) * 128], ident[:])
                                nc.vector.tensor_copy(
                                    stage[:, toff * 256 + fo * 128: toff * 256 + (fo + 1) * 128],
                                    tp[:])
                        if (b0 + bs) // 128 == TA:
                            # big region done: start its AllGather early
                            ccs.append(ag_chunk(i + 1, row_stage2A, 0))
                hT = [hTn0, hTn1]
                if not last:
                    ccs.append(ag_chunk(i + 1, row_stage2B, 1))

            # ---- final local max ----
            lm = stat.tile([128, 2], fp32)
            nc.vector.reduce_max(lm[:, 0:1], hT[0][:], axis=mybir.AxisListType.X)
            nc.vector.reduce_max(lm[:, 1:2], hT[1][:], axis=mybir.AxisListType.X)
            nc.sync.dma_start(locmax_d[:], lm[:])

    # the gather reads rows >= NC_*ROWS_A through tabA's base address, so
    # tabB must sit immediately after tabA in the shared scratchpad
    for ta, tb in _tab_pairs:
        a = nc.lookup_mls(ta.opt().tensor).memorylocations[0].addr
        b = nc.lookup_mls(tb.opt().tensor).memorylocations[0].addr
        assert b == a + NC_ * ROWS_A * D * 2, (a, b)
    nc.compile()
    return nc


def kernel(**inputs) -> np.ndarray:
    per_core, p_tile = _prep(inputs)
    if p_tile not in _nc_cache:
        _nc_cache[p_tile] = _build(p_tile)
    nc = _nc_cache[p_tile]
    in_maps = [
        dict(idx=pc["idx"], dloc=pc["dloc"], iota=pc["iota"], degb=pc["degb"],
             nodesT=pc["nodesT"], onehot=pc["onehot"], vmask=pc["vmask"],
             wf=pc["wf"], cvec=pc["cvec"], bupd=pc["bupd"], te=pc["te"],
             wproj=pc["wproj"], ident=pc["ident"])
        for pc in per_core
    ]
    res = bass_utils.run_bass_kernel_spmd(nc, in_maps, list(range(NC_)), trace=False)
    lm = np.stack([res.results[c]["locmax"] for c in range(NC_)])  # [8,128,2]
    gmax = lm.max(axis=0).T.reshape(D)  # feat fo*128+p
    W_out = np.asarray(inputs["W_out"], np.float32)
    b_out = np.asarray(inputs["b_out"], np.float32)
    return (gmax @ W_out + b_out).astype(np.float32)
